# revision 4
# baseline (speedup 1.0000x reference)
import sys

sys.path.insert(0, "/opt/trn_rl_repo")
import numpy as np
import concourse.bass as bass
import concourse.tile as tile
from concourse import bacc, mybir
from concourse.bass_utils import run_bass_kernel_spmd

F32 = mybir.dt.float32
F32R = mybir.dt.float32r
AF = mybir.ActivationFunctionType

B, T, C = 64, 500, 256
E, H, D = 512, 8, 64
N_CORES = 8
BL = B // N_CORES  # batches per core

USE_F32R = False  # matmul dtype switch (f32r: 4x faster, ~2e-4 matmul rel err)

TT = [128, 128, 128, 116]  # t/s tile sizes (500 = 3*128 + 116)


def _mm(ap):
    return ap.bitcast(F32R) if USE_F32R else ap


def build_nc():
    nc = bacc.Bacc("TRN2", target_bir_lowering=False)
    xt = nc.dram_tensor("xt", [BL, C, T], F32, kind="ExternalInput")
    wat = nc.dram_tensor("wat", [C, 3 * E], F32, kind="ExternalInput")  # w_attn.T
    wpt = nc.dram_tensor("wpt", [E, E], F32, kind="ExternalInput")  # w_proj.T
    bqk = nc.dram_tensor("bqk", [128, 8], F32, kind="ExternalInput")
    bvb = nc.dram_tensor("bvb", [128, E], F32, kind="ExternalInput")
    bpb = nc.dram_tensor("bpb", [128, E], F32, kind="ExternalInput")
    out = nc.dram_tensor("out", [BL, T, E], F32, kind="ExternalOutput")

    with tile.TileContext(nc) as tc:
        _build_body(nc, tc, xt, wat, wpt, bqk, bvb, bpb, out)
    nc.compile()
    return nc


def _build_body(nc, tc, xt, wat, wpt, bqk, bvb, bpb, out):
    from contextlib import ExitStack

    ctx = ExitStack()
    with ctx:
        cpool = ctx.enter_context(tc.tile_pool(name="consts", bufs=1))
        xpool = ctx.enter_context(tc.tile_pool(name="x", bufs=2))
        qkpool = ctx.enter_context(tc.tile_pool(name="qk", bufs=2))
        vpool = ctx.enter_context(tc.tile_pool(name="v", bufs=2))
        epool = ctx.enter_context(tc.tile_pool(name="est", bufs=2))
        ypool = ctx.enter_context(tc.tile_pool(name="yt", bufs=2))
        opool = ctx.enter_context(tc.tile_pool(name="os", bufs=2))
        zpool = ctx.enter_context(tc.tile_pool(name="zr", bufs=3))
        # PSUM pools: 8 banks total
        ps_io = ctx.enter_context(tc.tile_pool(name="ps_io", bufs=2, space="PSUM"))
        ps_st = ctx.enter_context(tc.tile_pool(name="ps_st", bufs=3, space="PSUM"))
        ps_yt = ctx.enter_context(tc.tile_pool(name="ps_yt", bufs=2, space="PSUM"))
        ps_zb = ctx.enter_context(tc.tile_pool(name="ps_zb", bufs=1, space="PSUM"))

        # ---- constants ----
        wa = cpool.tile([128, 2 * 3 * E], F32, name="wa")  # 2 c-ktiles x [128,1536]
        for k in range(2):
            nc.sync.dma_start(wa[:, k * 1536:(k + 1) * 1536], wat[k * 128:(k + 1) * 128, :])
        wp = cpool.tile([128, 4 * E], F32, name="wp")  # 4 e-ktiles x [128,512]
        for k in range(4):
            nc.sync.dma_start(wp[:, k * E:(k + 1) * E], wpt[k * 128:(k + 1) * 128, :])
        bqk_t = cpool.tile([128, 8], F32, name="bqk_t")
        nc.sync.dma_start(bqk_t[:], bqk[:, :])
        bvb_t = cpool.tile([128, E], F32, name="bvb_t")
        nc.sync.dma_start(bvb_t[:], bvb[:, :])
        bpb_t = cpool.tile([128, E], F32, name="bpb_t")
        nc.sync.dma_start(bpb_t[:], bpb[:, :])
        ones_t = cpool.tile([1, 64], F32, name="ones_t")
        nc.vector.memset(ones_t[:], 1.0)

        for b in range(BL):
            # ---- load xT(b): [256, 500] as [128, 2*500] ----
            xtb = xpool.tile([128, 2 * T], F32, name=f"xtb{b}", tag="xtb")
            for k in range(2):
                nc.sync.dma_start(xtb[:, k * T:(k + 1) * T], xt[b, k * 128:(k + 1) * 128, :])

            # ---- qkT(b): [1024(e), 500(t)] as [128, 8*500], e-tile j in cols ----
            qk = qkpool.tile([128, 8 * T], F32, name=f"qk{b}", tag="qk")
            for m in range(8):
                pq = ps_io.tile([128, T], F32, name=f"pq{b}_{m}", tag="ps_io")
                for k in range(2):
                    nc.tensor.matmul(
                        pq[:],
                        _mm(wa[:, k * 1536 + m * 128:k * 1536 + (m + 1) * 128]),
                        _mm(xtb[:, k * T:(k + 1) * T]),
                        start=(k == 0), stop=(k == 1),
                    )
                # add per-partition bias (b_attn for q/k) while copying to SBUF
                nc.scalar.activation(qk[:, m * T:(m + 1) * T], pq[:], AF.Identity,
                                     bias=bqk_t[:, m:m + 1])

            # ---- v(b): [500(t), 512(e)] + bias, into v_aug [128, 4*520] ----
            va = vpool.tile([128, 4 * 520], F32, name=f"va{b}", tag="va")
            for mt in range(4):
                tt = TT[mt]
                pv = ps_io.tile([128, E], F32, name=f"pv{b}_{mt}", tag="ps_io")
                for k in range(2):
                    nc.tensor.matmul(
                        pv[0:tt, :],
                        _mm(xtb[:, k * T + mt * 128:k * T + mt * 128 + tt]),
                        _mm(wa[:, k * 1536 + 1024:k * 1536 + 1536]),
                        start=(k == 0), stop=(k == 1),
                    )
                va3 = va[:, mt * 520:(mt + 1) * 520].rearrange("p (h m) -> p h m", h=H)
                nc.vector.tensor_add(
                    va3[0:tt, :, 0:64],
                    pv[0:tt, :].rearrange("p (h m) -> p h m", h=H),
                    bvb_t[0:tt, :].rearrange("p (h m) -> p h m", h=H),
                )
                nc.vector.memset(va3[:, :, 64:65], 1.0)

            # ---- attention heads, z-chain pipelined 2 deep ----
            est = {}
            pyt = {}
            zr = {}
            zb = {}
            ysb = {}

            def do_st(h):
                # ST[s,t] = k_h @ q_h^T; exp(ST/8) -> est[h]
                jq, oq = h // 2, (h % 2) * 64
                jk, ok = 4 + h // 2, (h % 2) * 64
                e = epool.tile([128, 4 * T], F32, name=f"est{b}_{h}", tag="est")
                est[h] = e
                for s in range(4):
                    st = TT[s]
                    pst = ps_st.tile([128, T], F32, name=f"pst{b}_{h}_{s}", tag="ps_st")
                    nc.tensor.matmul(
                        pst[0:st, :],
                        _mm(qk[ok:ok + 64, jk * T + s * 128:jk * T + s * 128 + st]),
                        _mm(qk[oq:oq + 64, jq * T:(jq + 1) * T]),
                        start=True, stop=True,
                    )
                    nc.scalar.activation(e[0:st, s * T:(s + 1) * T], pst[0:st, :],
                                         AF.Exp, scale=0.125)

            def do_yt(h):
                # yT[d,t] (+ z in row 64) = [v_h | 1]^T @ expST
                p = ps_yt.tile([65, T], F32, name=f"pyt{b}_{h}", tag="ps_yt")
                pyt[h] = p
                e = est[h]
                for s in range(4):
                    st = TT[s]
                    nc.tensor.matmul(
                        p[:],
                        _mm(va[0:st, s * 520 + 65 * h:s * 520 + 65 * h + 65]),
                        _mm(e[0:st, s * T:(s + 1) * T]),
                        start=(s == 0), stop=(s == 3),
                    )
                z = zpool.tile([1, T], F32, name=f"zr{b}_{h}", tag="zr")
                zr[h] = z
                nc.vector.reciprocal(z[:], p[64:65, :])
                ys = zpool.tile([64, T], F32, name=f"ysb{b}_{h}", tag="ysb")
                ysb[h] = ys
                nc.scalar.copy(ys[:], p[0:64, :])

            def do_zb(h):
                p = ps_zb.tile([64, T], F32, name=f"pzb{b}_{h}", tag="ps_zb")
                zb[h] = p
                nc.tensor.matmul(p[:], _mm(ones_t[:]), _mm(zr[h][:]), start=True, stop=True)

            def do_norm(h, yt):
                j, o = h // 2, (h % 2) * 64
                nc.vector.tensor_mul(
                    yt[o:o + 64, j * T:(j + 1) * T], ysb[h][:], zb[h][:]
                )

            yt = ypool.tile([128, 4 * T], F32, name=f"yt{b}", tag="yt")
            for h in range(H):
                do_st(h)
                if h >= 1:
                    do_yt(h - 1)
                if h >= 2:
                    do_zb(h - 2)
                    do_norm(h - 2, yt)
            do_yt(H - 1)
            for h in (H - 2, H - 1):
                do_zb(h)
                do_norm(h, yt)

            # ---- proj(b): out[t,f] = yT^T @ wpT + bproj ----
            osb = opool.tile([128, 4 * E], F32, name=f"osb{b}", tag="osb")
            for mt in range(4):
                tt = TT[mt]
                po = ps_io.tile([128, E], F32, name=f"po{b}_{mt}", tag="ps_io")
                for k in range(4):
                    nc.tensor.matmul(
                        po[0:tt, :],
                        _mm(yt[:, k * T + mt * 128:k * T + mt * 128 + tt]),
                        _mm(wp[:, k * E:(k + 1) * E]),
                        start=(k == 0), stop=(k == 3),
                    )
                nc.vector.tensor_add(osb[0:tt, mt * E:(mt + 1) * E], po[0:tt, :], bpb_t[0:tt, :])
                nc.sync.dma_start(out[b, mt * 128:mt * 128 + tt, :], osb[0:tt, mt * E:(mt + 1) * E])


_NC = None


def _get_nc():
    global _NC
    if _NC is None:
        _NC = build_nc()
    return _NC


def prep_inputs(x, w_attn, b_attn, w_proj, b_proj):
    x = np.asarray(x, np.float32)
    w_attn = np.asarray(w_attn, np.float32)
    b_attn = np.asarray(b_attn, np.float32)
    w_proj = np.asarray(w_proj, np.float32)
    b_proj = np.asarray(b_proj, np.float32)

    xt_all = np.ascontiguousarray(x.transpose(0, 2, 1))  # [B, C, T]
    wat = np.ascontiguousarray(w_attn.T)  # [C, 1536]
    wpt = np.ascontiguousarray(w_proj.T)  # [E, E]
    bqk = np.ascontiguousarray(b_attn[:1024].reshape(8, 128).T)  # [128, 8]
    bvb = np.ascontiguousarray(np.tile(b_attn[1024:1536][None, :], (128, 1)))
    bpb = np.ascontiguousarray(np.tile(b_proj[None, :], (128, 1)))

    in_maps = []
    for c in range(N_CORES):
        in_maps.append({
            "xt": np.ascontiguousarray(xt_all[c * BL:(c + 1) * BL]),
            "wat": wat, "wpt": wpt, "bqk": bqk, "bvb": bvb, "bpb": bpb,
        })
    return in_maps


def kernel(x, w_attn, b_attn, w_proj, b_proj):
    nc = _get_nc()
    in_maps = prep_inputs(x, w_attn, b_attn, w_proj, b_proj)
    res = run_bass_kernel_spmd(nc, in_maps, core_ids=list(range(N_CORES)))
    out = np.concatenate([res.results[c]["out"] for c in range(N_CORES)], axis=0)
    return out.astype(np.float32)


# revision 18
# speedup vs baseline: 5698.2087x; 5698.2087x over previous
import sys

sys.path.insert(0, "/opt/trn_rl_repo")
import numpy as np
import concourse.bass as bass
import concourse.tile as tile
from concourse import bacc, mybir
from concourse.bass_utils import run_bass_kernel_spmd

F32 = mybir.dt.float32
F32R = mybir.dt.float32r
AF = mybir.ActivationFunctionType

B, T, C = 64, 500, 256
E, H, D = 512, 8, 64
N_CORES = 8
BL = B // N_CORES  # batches per core

USE_F32R = False  # matmul dtype switch (f32r: 4x faster, ~2e-4 matmul rel err)

TT = [128, 128, 128, 116]  # t/s tile sizes (500 = 3*128 + 116)


MM_DT = F32R if USE_F32R else F32


def _mm(ap):
    # reinterpret as the matmul dtype (no-op when already MM_DT / fp32 mode)
    return ap if ap.dtype == MM_DT else ap.bitcast(MM_DT)


def build_nc():
    nc = bacc.Bacc("TRN2", target_bir_lowering=False)
    xt = nc.dram_tensor("xt", [BL, C, T], F32, kind="ExternalInput")
    wat = nc.dram_tensor("wat", [C, 3 * E], F32, kind="ExternalInput")  # w_attn.T
    wpt = nc.dram_tensor("wpt", [E, E], F32, kind="ExternalInput")  # w_proj.T
    bqk = nc.dram_tensor("bqk", [128, 8], F32, kind="ExternalInput")
    bvb = nc.dram_tensor("bvb", [128, E], F32, kind="ExternalInput")
    bpb = nc.dram_tensor("bpb", [128, E], F32, kind="ExternalInput")
    out = nc.dram_tensor("out", [BL, T, E], F32, kind="ExternalOutput")

    with tile.TileContext(nc) as tc:
        _build_body(nc, tc, xt, wat, wpt, bqk, bvb, bpb, out)
    nc.compile()
    return nc


def _build_body(nc, tc, xt, wat, wpt, bqk, bvb, bpb, out):
    from contextlib import ExitStack

    ctx = ExitStack()
    with ctx:
        cpool = ctx.enter_context(tc.tile_pool(name="consts", bufs=1))
        xpool = ctx.enter_context(tc.tile_pool(name="x", bufs=2))
        qkpool = ctx.enter_context(tc.tile_pool(name="qk", bufs=2))
        vpool = ctx.enter_context(tc.tile_pool(name="v", bufs=2))
        epool = ctx.enter_context(tc.tile_pool(name="est", bufs=4))
        ypool = ctx.enter_context(tc.tile_pool(name="yt", bufs=2))
        opool = ctx.enter_context(tc.tile_pool(name="os", bufs=2))
        zpool = ctx.enter_context(tc.tile_pool(name="zr", bufs=3))
        # PSUM pools: 8 banks total
        ps_io = ctx.enter_context(tc.tile_pool(name="ps_io", bufs=2, space="PSUM"))
        ps_st = ctx.enter_context(tc.tile_pool(name="ps_st", bufs=4, space="PSUM"))
        ps_yt = ctx.enter_context(tc.tile_pool(name="ps_yt", bufs=2, space="PSUM"))

        # ---- constants ----
        wa = cpool.tile([128, 2 * 3 * E], MM_DT, name="wa")  # 2 c-ktiles x [128,1536]
        for k in range(2):
            nc.sync.dma_start(wa[:, k * 1536:(k + 1) * 1536], _mm(wat[k * 128:(k + 1) * 128, :]))
        wp = cpool.tile([128, 4 * E], MM_DT, name="wp")  # 4 e-ktiles x [128,512]
        for k in range(4):
            nc.sync.dma_start(wp[:, k * E:(k + 1) * E], _mm(wpt[k * 128:(k + 1) * 128, :]))
        bqk_t = cpool.tile([128, 8], F32, name="bqk_t")
        nc.sync.dma_start(bqk_t[:], bqk[:, :])
        bvb_t = cpool.tile([128, E], F32, name="bvb_t")
        nc.sync.dma_start(bvb_t[:], bvb[:, :])
        bpb_t = cpool.tile([128, E], F32, name="bpb_t")
        nc.sync.dma_start(bpb_t[:], bpb[:, :])
        ones8 = cpool.tile([128, 8], F32, name="ones8")
        nc.vector.memset(ones8[:], 1.0)

        def do_proj(yt, b):
            # out[t,f] = yT^T @ wpT + bproj
            osb = opool.tile([128, 4 * E], F32, name=f"osb{b}", tag="osb")
            for mt in range(4):
                tt = TT[mt]
                po = ps_io.tile([128, E], F32, name=f"po{b}_{mt}", tag="ps_io")
                for k in range(4):
                    nc.tensor.matmul(
                        po[0:tt, :],
                        _mm(yt[:, k * T + mt * 128:k * T + mt * 128 + tt]),
                        _mm(wp[:, k * E:(k + 1) * E]),
                        start=(k == 0), stop=(k == 3),
                    )
                nc.vector.tensor_add(osb[0:tt, mt * E:(mt + 1) * E], po[0:tt, :], bpb_t[0:tt, :])
                nc.sync.dma_start(out[b, mt * 128:mt * 128 + tt, :], osb[0:tt, mt * E:(mt + 1) * E])

        def load_x(b):
            xtb = xpool.tile([128, 2 * T], MM_DT, name=f"xtb{b}", tag="xtb")
            for k in range(2):
                nc.sync.dma_start(xtb[:, k * T:(k + 1) * T], _mm(xt[b, k * 128:(k + 1) * 128, :]))
            return xtb

        xtb_next = load_x(0)
        prev = None  # (yt, b) awaiting projection
        for b in range(BL):
            xtb = xtb_next

            # ---- qkT(b): [1024(e), 500(t)] as [128, 8*500], e-tile j in cols ----
            qk = qkpool.tile([128, 8 * T], MM_DT, name=f"qk{b}", tag="qk")
            for m in range(8):
                pq = ps_io.tile([128, T], F32, name=f"pq{b}_{m}", tag="ps_io")
                for k in range(2):
                    nc.tensor.matmul(
                        pq[:],
                        _mm(wa[:, k * 1536 + m * 128:k * 1536 + (m + 1) * 128]),
                        _mm(xtb[:, k * T:(k + 1) * T]),
                        start=(k == 0), stop=(k == 1),
                    )
                # add per-partition bias (b_attn for q/k) while copying to SBUF
                nc.scalar.activation(qk[:, m * T:(m + 1) * T], pq[:], AF.Identity,
                                     bias=bqk_t[:, m:m + 1])

            # ---- v(b): [500(t), 512(e)] + bias, into v_aug [128, 4*520] ----
            va = vpool.tile([128, 4 * 520], MM_DT, name=f"va{b}", tag="va")
            for mt in range(4):
                tt = TT[mt]
                pv = ps_io.tile([128, E], F32, name=f"pv{b}_{mt}", tag="ps_io")
                for k in range(2):
                    nc.tensor.matmul(
                        pv[0:tt, :],
                        _mm(xtb[:, k * T + mt * 128:k * T + mt * 128 + tt]),
                        _mm(wa[:, k * 1536 + 1024:k * 1536 + 1536]),
                        start=(k == 0), stop=(k == 1),
                    )
                va3 = va[:, mt * 520:(mt + 1) * 520].rearrange("p (h m) -> p h m", h=H)
                nc.vector.tensor_add(
                    va3[0:tt, :, 0:64],
                    pv[0:tt, :].rearrange("p (h m) -> p h m", h=H),
                    bvb_t[0:tt, :].rearrange("p (h m) -> p h m", h=H),
                )
                nc.vector.tensor_copy(
                    va3[:, :, 64:65], ones8[:].rearrange("p (h o) -> p h o", o=1)
                )

            if b + 1 < BL:
                xtb_next = load_x(b + 1)

            if prev is not None:
                do_proj(*prev)

            # ---- attention heads, z-chain pipelined 2 deep ----
            est = {}
            pyt = {}
            zr = {}
            zbs = {}

            def do_st_pair(h0, h1):
                # ST[s,t] = k_h @ q_h^T; exp(ST/8) -> est[h]
                # heads h0 (rows 0:64) and h1 (rows 64:128) occupy disjoint PE
                # row groups -> their matmuls run concurrently in the array
                for h in (h0, h1):
                    est[h] = epool.tile([128, 4 * T], MM_DT, name=f"est{b}_{h}", tag="est")
                for s in range(4):
                    st = TT[s]
                    for h in (h0, h1):
                        jq, oq = h // 2, (h % 2) * 64
                        jk, ok = 4 + h // 2, (h % 2) * 64
                        pst = ps_st.tile([128, T], F32, name=f"pst{b}_{h}_{s}", tag="ps_st")
                        nc.tensor.matmul(
                            pst[0:st, :],
                            _mm(qk[ok:ok + 64, jk * T + s * 128:jk * T + s * 128 + st]),
                            _mm(qk[oq:oq + 64, jq * T:(jq + 1) * T]),
                            start=True, stop=True,
                        )
                        nc.scalar.activation(est[h][0:st, s * T:(s + 1) * T], pst[0:st, :],
                                             AF.Exp, scale=0.125)

            def do_yt(h):
                # yT[d,t] (+ z in row 64) = [v_h | 1]^T @ expST
                p = ps_yt.tile([65, T], F32, name=f"pyt{b}_{h}", tag="ps_yt")
                pyt[h] = p
                e = est[h]
                for s in range(4):
                    st = TT[s]
                    nc.tensor.matmul(
                        p[:],
                        _mm(va[0:st, s * 520 + 65 * h:s * 520 + 65 * h + 65]),
                        _mm(e[0:st, s * T:(s + 1) * T]),
                        start=(s == 0), stop=(s == 3),
                    )
                z = zpool.tile([1, T], F32, name=f"zr{b}_{h}", tag="zr")
                zr[h] = z
                nc.vector.reciprocal(z[:], p[64:65, :])
                zs = zpool.tile([64, T], F32, name=f"zbs{b}_{h}", tag="zbs")
                zbs[h] = zs
                nc.gpsimd.partition_broadcast(zs[:], z[:])

            def do_norm(h, yt):
                j, o = h // 2, (h % 2) * 64
                nc.vector.tensor_mul(
                    yt[o:o + 64, j * T:(j + 1) * T], pyt[h][0:64, :], zbs[h][:]
                )

            yt = ypool.tile([128, 4 * T], MM_DT, name=f"yt{b}", tag="yt")
            for hp in range(H // 2):
                do_st_pair(2 * hp, 2 * hp + 1)
                if hp >= 1:
                    for h in (2 * hp - 2, 2 * hp - 1):
                        do_yt(h)
                        do_norm(h, yt)
            for h in (H - 2, H - 1):
                do_yt(h)
                do_norm(h, yt)

            prev = (yt, b)
        do_proj(*prev)


_NC = None


def _get_nc():
    global _NC
    if _NC is None:
        _NC = build_nc()
    return _NC


def prep_inputs(x, w_attn, b_attn, w_proj, b_proj):
    x = np.asarray(x, np.float32)
    w_attn = np.asarray(w_attn, np.float32)
    b_attn = np.asarray(b_attn, np.float32)
    w_proj = np.asarray(w_proj, np.float32)
    b_proj = np.asarray(b_proj, np.float32)

    xt_all = np.ascontiguousarray(x.transpose(0, 2, 1))  # [B, C, T]
    wat = np.ascontiguousarray(w_attn.T)  # [C, 1536]
    wpt = np.ascontiguousarray(w_proj.T)  # [E, E]
    bqk = np.ascontiguousarray(b_attn[:1024].reshape(8, 128).T)  # [128, 8]
    bvb = np.ascontiguousarray(np.tile(b_attn[1024:1536][None, :], (128, 1)))
    bpb = np.ascontiguousarray(np.tile(b_proj[None, :], (128, 1)))

    in_maps = []
    for c in range(N_CORES):
        in_maps.append({
            "xt": np.ascontiguousarray(xt_all[c * BL:(c + 1) * BL]),
            "wat": wat, "wpt": wpt, "bqk": bqk, "bvb": bvb, "bpb": bpb,
        })
    return in_maps


def kernel(x, w_attn, b_attn, w_proj, b_proj):
    nc = _get_nc()
    in_maps = prep_inputs(x, w_attn, b_attn, w_proj, b_proj)
    res = run_bass_kernel_spmd(nc, in_maps, core_ids=list(range(N_CORES)))
    out = np.concatenate([res.results[c]["out"] for c in range(N_CORES)], axis=0)
    return out.astype(np.float32)
